# revision 62
# baseline (speedup 1.0000x reference)
"""CheckersGPT dense transformer forward pass on 8 Trainium2 NeuronCores.

Strategy: pure data-parallel over the batch dim (16 batches -> 2 per core).
Each core runs the full 6-layer transformer on its 512 tokens (2 batches x
256 tokens); no collectives, outputs concatenated on the host.

Key restructuring vs a direct translation (each head uses a full ExE Q/K/V):
  - Host folds Wqk = Wq @ Wk^T and Wvo_h = Wv_h @ Wo_h. The K projection and
    the H*E->E output projection disappear: energy = (x Wqk) x^T and
    attn_out = sum_h att_h (x Wvo_h).
  - LN1's affine params are folded into ff1 (exact); LN2's scale is folded
    into the next layer's Wqk/Wvo/wout (exact when ln2_b == 0, which holds
    for this model); bo is folded into the h==0 attention accumulation.
  - The two batches per core are INDEPENDENT sequences, so each layer's
    attention is split into two "waves" (batch 0, batch 1). The FFN +
    layernorm chains of wave b run on DVE/Act/Pool underneath the ~38us of
    PE projection work of the opposite wave, so the PE never idles at layer
    boundaries. Each FFN token-chunk is emitted as a PE-free part A
    (residual + LN1 + transposed-h1) right after its wave's last tail, and
    a part B (ff1/ff2/LN2/new-x) two heads into the opposite wave.
  - Energy is computed TRANSPOSED ([j, i] = key-major) with xT as lhsT, so
    att @ V produces attn_out in natural token-major layout directly. Softmax
    runs without max-subtraction (energies are bounded by the 0.02-scale
    weights); denominators come from N=1 matmuls against a ones vector, and
    1/den is applied by a fused (U * rec) + acc scalar_tensor_tensor.
  - All natural->transposed layout changes for matmul feeds go through the
    DMA xbar transpose engine (dma_start_transpose, bf16), not the PE.
  - 1/sqrt(var+eps) is computed as exp(-0.5*ln(var+eps)): ln/exp/relu/copy
    live in one activation table set, so the Act engine never swaps tables.

Layout per core (P=128 partitions):
  xT   [128, TC, EC, 128] : x transposed; [p, tcc, ec, t] = token chunk tcc,
                            embed dim ec*128+p, token t. matmul lhsT/rhs.
  xN   [128, TC, E]       : x natural; residuals / layernorm.
All matmuls are out = lhsT.T @ rhs with contraction on the partition dim.
The last layer only computes attention/FFN for the final token of each batch.
"""

import os
import numpy as np
from contextlib import ExitStack

import ml_dtypes
import concourse.bass as bass
import concourse.tile as tile
from concourse import bacc, mybir
from concourse.bass_utils import run_bass_kernel_spmd

F32 = mybir.dt.float32
BF16 = mybir.dt.bfloat16
I32 = mybir.dt.int32
AX = mybir.AxisListType
ALU = mybir.AluOpType
ACTF = mybir.ActivationFunctionType

V, E, L, H, B, T = 512, 512, 6, 8, 16, 256
NCORES = 8
BPC = B // NCORES          # batches per core
TOK = BPC * T              # tokens per core
P = 128
EC = E // P                # embed chunks of 128
TC = TOK // P              # token chunks of 128
NEG = -1e9
EPS = 1e-5

MM_DT = BF16
F8 = mybir.dt.float8e4
NP_WDT = ml_dtypes.bfloat16
NP_F8 = ml_dtypes.float8_e4m3
SW8 = 64.0                 # host pre-scale on the fp8 Wqk weights
W_BUFS = 11                # weight tiles in flight per kind (1 per head each)

_CACHE = {}


def _mm(nc, out, lhsT, rhs, start, stop, skip=False):
    nc.tensor.matmul(out, lhsT, rhs, start=start, stop=stop,
                     skip_group_check=skip)


def _mm8(nc, out, lhsT, rhs, start, stop):
    nc.tensor.matmul(out, lhsT, rhs, start=start, stop=stop,
                     perf_mode=mybir.MatmulPerfMode.DoubleRow)


def _build(nlayers=L, reps=1, last_opt=True, dev_affine=False):
    nc = bacc.Bacc("TRN2", target_bir_lowering=False, debug=False, num_devices=NCORES)

    def din(name, shape, dtype=F32):
        return nc.dram_tensor(name, list(shape), dtype, kind="ExternalInput").ap()

    tok = din("tok", [P, TC], I32)            # token ids, p-major within chunks
    emb = din("emb", [V, E])
    pe2 = din("pe2", [TOK, E], MM_DT)         # positional encoding tiled over BPC
    wqk = din("wqk", [L, H, E, E], F8)        # 64 * Wq @ Wk^T (ln2 scale folded)
    wvo = din("wvo", [L, H, E, E], MM_DT)     # Wv @ Wo_h   (ln2 scale folded)
    bo = din("bo", [L, E])
    ln2w = din("ln2w", [L, E])
    ln2b = din("ln2b", [L, E])
    ff1w = din("ff1w", [L, E, E], MM_DT)      # ln1_w/ln1_b pre-folded on host
    ff1b = din("ff1b", [L, E])
    ff2w = din("ff2w", [L, E, E], MM_DT)
    ff2b = din("ff2b", [L, E])
    wout = din("wout", [E, V], MM_DT)         # ln2 scale of last layer folded
    bout = din("bout", [V])
    maskd = din("maskd", [P, P])              # additive causal mask, diag block
    ident = din("ident", [P, P])
    probs = nc.dram_tensor("probs", [BPC, V], F32, kind="ExternalOutput").ap()
    aps = (emb, pe2, wqk, wvo, bo, ln2w, ln2b,
           ff1w, ff1b, ff2w, ff2b, wout, bout, maskd, ident, probs, tok)

    with tile.TileContext(nc) as tc, ExitStack() as ctx:
        if reps > 1:
            with tc.For_i(0, reps, 1):
                _emit(nc, tc, ctx, aps, nlayers, last_opt, dev_affine)
        else:
            _emit(nc, tc, ctx, aps, nlayers, last_opt, dev_affine)

    nc.compile()
    return nc


def _emit(nc, tc, ctx, aps, nlayers, last_opt, dev_affine):
    (emb, pe2, wqk, wvo, bo, ln2w, ln2b,
     ff1w, ff1b, ff2w, ff2b, wout, bout, maskd, ident, probs, tok) = aps
    ep = ctx.enter_context

    const = ep(tc.tile_pool(name="const", bufs=1))
    wq_p = ep(tc.tile_pool(name="wq", bufs=W_BUFS))
    wff_p = ep(tc.tile_pool(name="wff", bufs=2))
    bias_p = ep(tc.tile_pool(name="bias", bufs=2))
    act_p = ep(tc.tile_pool(name="act", bufs=2))
    qkv_p = ep(tc.tile_pool(name="qkvact", bufs=2))
    ff_p = ep(tc.tile_pool(name="ffact", bufs=2))
    tmp_p = ep(tc.tile_pool(name="tmp", bufs=2))
    esb_p = ep(tc.tile_pool(name="esb", bufs=3))
    st_p = ep(tc.tile_pool(name="stats", bufs=4))
    out_p = ep(tc.tile_pool(name="outp", bufs=1))

    ppb = ep(tc.tile_pool(name="ppb", bufs=3, space="PSUM"))
    ppg = ep(tc.tile_pool(name="ppg", bufs=3, space="PSUM"))
    ppe = ep(tc.tile_pool(name="ppe", bufs=2, space="PSUM"))

    def wtile(pool, dram2d, tag, bufs=None, dtype=MM_DT):
        t = pool.tile([P, EC, E], dtype, tag=tag, bufs=bufs)
        nc.sync.dma_start(
            out=t[:], in_=dram2d.rearrange("(c p) o -> p c o", p=P)
        )
        return t

    def bbcast(vec_ap, tag):
        t = bias_p.tile([P, E], F32, tag=tag)
        nc.sync.dma_start(out=t[:], in_=vec_ap.partition_broadcast(P))
        return t

    # ---- constants; tok first (the embedding gather gates the whole start),
    # then layer-0 head-0 weights ahead of the bulk constants so the first
    # GT matmuls are not starved behind the 1MB positional-encoding DMA ----
    tok_t = const.tile([P, TC], I32)
    nc.sync.dma_start(out=tok_t[:], in_=tok)
    eps_t = const.tile([P, 1], F32)
    nc.vector.memset(eps_t[:], EPS)
    ones_t = const.tile([P, 1], MM_DT)
    nc.vector.memset(ones_t[:], 1.0)
    # startup order on the SP queue: wqk00, pe2-chunk0, wvo00, pe2 rest,
    # maskd, ident — so the first gather chunk's x tile and the first head's
    # weights are both ready ~5us in. The Act hwdge queue stays empty so the
    # init xbar transposes issue with no backlog.
    wqk00 = wtile(wq_p, wqk[0, 0], "w8", dtype=F8)
    pe_t = const.tile([P, TC, E], MM_DT)
    pe2r = pe2.rearrange("(c p) o -> p c o", p=P)
    nc.sync.dma_start(out=pe_t[:, 0, :], in_=pe2r[:, 0, :])
    preW = {(0, 0): (wqk00, wtile(wq_p, wvo[0, 0], "w"))}
    for c in range(1, TC):
        nc.sync.dma_start(out=pe_t[:, c, :], in_=pe2r[:, c, :])
    maskd_t = const.tile([P, P], F32)
    nc.sync.dma_start(out=maskd_t[:], in_=maskd)
    ident_t = const.tile([P, P], F32)
    nc.sync.dma_start(out=ident_t[:], in_=ident)

    def evac(dst, src, use_act, scale=None):
        """PSUM -> SBUF copy (dtype conversion happens on write)."""
        if use_act:
            if scale is None:
                nc.scalar.copy(dst, src)
            else:
                nc.scalar.activation(out=dst, in_=src, func=ACTF.Copy,
                                     scale=scale)
        else:
            if scale is None:
                nc.vector.tensor_copy(dst, src)
            else:
                nc.vector.tensor_scalar(out=dst, in0=src, scalar1=scale,
                                        scalar2=None, op0=ALU.mult,
                                        op1=ALU.bypass)

    def ln_stats(src, tag, rows=P):
        """mean + 1/sqrt(var+eps) of src [rows, E].

        rsqrt via the bit-trick + 2 Newton steps, entirely on DVE: keeps
        sqrt/ln off the Act engine so it never reloads its function table
        (exp/relu/copy all live in the one 'exp_and_others' set).
        """
        stt = st_p.tile([P, 6], F32, tag=tag + "s")
        nc.vector.bn_stats(out=stt[:rows], in_=src)
        mv = st_p.tile([P, 2], F32, tag=tag + "m")
        nc.vector.bn_aggr(out=mv[:rows], in_=stt[:rows])
        vp = st_p.tile([P, 1], F32, tag=tag + "v")
        nc.vector.tensor_scalar(
            out=vp[:rows], in0=mv[:rows, 1:2], scalar1=EPS, scalar2=None,
            op0=ALU.add, op1=ALU.bypass,
        )
        hh = st_p.tile([P, 1], F32, tag=tag + "h")
        nc.vector.tensor_scalar(
            out=hh[:rows], in0=vp[:rows], scalar1=-0.5, scalar2=None,
            op0=ALU.mult, op1=ALU.bypass,
        )
        rs = st_p.tile([P, 1], F32, tag=tag + "r")
        nc.vector.tensor_scalar(
            out=rs[:rows].bitcast(I32), in0=vp[:rows].bitcast(I32),
            scalar1=1, scalar2=None,
            op0=ALU.logical_shift_right, op1=ALU.bypass,
        )
        nc.vector.tensor_scalar(
            out=rs[:rows].bitcast(I32), in0=rs[:rows].bitcast(I32),
            scalar1=-1, scalar2=0x5F3759DF, op0=ALU.mult, op1=ALU.add,
        )
        for it in range(2):
            t0 = st_p.tile([P, 1], F32, tag=tag + "n", name=f"nw{tag}{it}")
            nc.vector.tensor_mul(out=t0[:rows], in0=rs[:rows], in1=rs[:rows])
            nc.vector.tensor_scalar(
                out=t0[:rows], in0=t0[:rows], scalar1=hh[:rows, 0:1],
                scalar2=1.5, op0=ALU.mult, op1=ALU.add,
            )
            nc.vector.tensor_mul(out=rs[:rows], in0=rs[:rows], in1=t0[:rows])
        return mv, rs

    def layernorm(src, dst, w_b, b_b, tag, rows=P):
        mv, rs = ln_stats(src, tag, rows)
        nc.vector.tensor_scalar(
            out=dst, in0=src, scalar1=mv[:rows, 0:1], scalar2=rs[:rows],
            op0=ALU.subtract, op1=ALU.mult,
        )
        if w_b is not None:
            nc.gpsimd.tensor_mul(out=dst, in0=dst, in1=w_b[:rows, :])
            nc.gpsimd.tensor_add(out=dst, in0=dst, in1=b_b[:rows, :])

    # ---- embedding gather + positional encoding ----
    xN = act_p.tile([P, TC, E], F32, tag="xN")
    for c in range(TC):
        nc.gpsimd.indirect_dma_start(
            out=xN[:, c, :], out_offset=None, in_=emb,
            in_offset=bass.IndirectOffsetOnAxis(ap=tok_t[:, c : c + 1], axis=0),
        )
    xT = act_p.tile([P, TC, EC, P], MM_DT, tag="xT")
    xT8 = act_p.tile([P, TC, EC, P], F8, tag="xT8")
    for c in range(TC):
        nc.vector.tensor_add(out=xN[:, c, :], in0=xN[:, c, :], in1=pe_t[:, c, :])
        xbf0 = ff_p.tile([P, E], MM_DT, tag="xbf")
        nc.gpsimd.tensor_copy(xbf0[:], xN[:, c, :])
        nc.scalar.dma_start_transpose(out=xT[:, c, :, :], in_=xbf0[:])
        nc.gpsimd.tensor_copy(xT8[:, c, :, :], xT[:, c, :, :])

    carry = None       # tail(7) of the previous wave + its FFN part-A chunks
    pendingB = []      # FFN part-B chunks of the previous wave

    for l in range(nlayers):
        last = last_opt and (l == L - 1) and (nlayers == L)
        biases = {}

        def load_biases(stage, l=l):
            # staggered so bias DMAs don't jam the queue ahead of head weights
            if stage == 1:
                biases["bo"] = bbcast(bo[l], "b_bo")
            elif stage == 2:
                t = bias_p.tile([P, EC], F32, tag="b_f1")
                nc.sync.dma_start(
                    out=t[:], in_=ff1b[l].rearrange("(c p) -> p c", p=P)
                )
                biases["ff1b"] = t
            elif stage == 3:
                biases["ff2b"] = bbcast(ff2b[l], "b_f2")
                if dev_affine:
                    biases["ln2w"] = bbcast(ln2w[l], "b_l2w")
                    biases["ln2b"] = bbcast(ln2b[l], "b_l2b")

        def tail(h, expT, Vp, w2, acc, den, biases=biases):
            """den + AV + normalized accumulation for one wave-head.

            The (U * 1/den) + acc update alternates between a fused DVE
            scalar_tensor_tensor and an Act scaled-copy + Pool add, so no
            single vector engine eats the whole attention tail."""
            _mm(nc, den[:, 0:1], expT[:, 0:P], ones_t[:, 0:1], True, True)
            U0 = ppb.tile([P, E], F32, tag="ppb")
            _mm(nc, U0[:], expT[:, 0:P], Vp[:, 0, :], True, True)
            _mm(nc, den[:, 1:2], expT[:, P : 2 * P], ones_t[:, 0:1], True, False)
            U1 = ppb.tile([P, E], F32, tag="ppb")
            _mm(nc, U1[:], expT[:, P : 2 * P], Vp[:, 0, :], True, False)
            _mm(nc, den[:, 1:2], expT[:, 2 * P : 3 * P], ones_t[:, 0:1], False, True)
            _mm(nc, U1[:], expT[:, 2 * P : 3 * P], Vp[:, 1, :], False, True)
            rec = st_p.tile([P, 2], F32, tag="rec", bufs=3)
            nc.vector.reciprocal(out=rec[:], in_=den[:])
            for ic, U in ((0, U0), (1, U1)):
                tcc = w2 + ic
                prev_ap = biases["bo"][:] if h == 0 else acc[:, tcc, :]
                if ic == 0:
                    nc.vector.scalar_tensor_tensor(
                        out=acc[:, tcc, :], in0=U[:], scalar=rec[:, ic : ic + 1],
                        in1=prev_ap, op0=ALU.mult, op1=ALU.add,
                    )
                else:
                    un = tmp_p.tile([P, E], F32, tag="un", bufs=3)
                    nc.scalar.activation(
                        out=un[:], in_=U[:], func=ACTF.Copy,
                        scale=rec[:, ic : ic + 1],
                    )
                    if h == 0:
                        nc.gpsimd.tensor_add(out=acc[:, tcc, :], in0=un[:],
                                             in1=prev_ap)
                    else:
                        nc.gpsimd.tensor_add(out=acc[:, tcc, :],
                                             in0=acc[:, tcc, :], in1=un[:])

        def make_ffn_A(tcc, acc, xN, h1Ts, biases):
            def chunkA():
                s1 = tmp_p.tile([P, E], F32, tag="s1")
                nc.vector.tensor_add(
                    out=s1[:], in0=acc[:, tcc, :], in1=xN[:, tcc, :]
                )
                mv, rs = ln_stats(s1[:], "ln1")
                h1bf = ff_p.tile([P, E], MM_DT, tag="h1bf")
                nc.vector.tensor_scalar(
                    out=h1bf[:], in0=s1[:], scalar1=mv[:, 0:1], scalar2=rs[:],
                    op0=ALU.subtract, op1=ALU.mult,
                )
                h1T = ff_p.tile([P, EC, P], MM_DT, tag="h1T")
                nc.scalar.dma_start_transpose(out=h1T[:], in_=h1bf[:])
                h1Ts[tcc] = h1T
            return chunkA

        def make_ffn_B(tcc, acc, xN_new, xT_new, xT8_new, h1Ts, biases, ffw):
            def chunkB():
                ff1w_t, ff2w_t = ffw[0]
                h1T = h1Ts[tcc]
                r1T = ff_p.tile([P, EC, P], MM_DT, tag="r1T")
                ps = ppg.tile([P, EC * P], F32, tag="ppg")
                for fc in range(EC):
                    for ec in range(EC):
                        _mm(nc, ps[:, fc * P : (fc + 1) * P],
                            ff1w_t[:, ec, fc * P : (fc + 1) * P],
                            h1T[:, ec, :], ec == 0, ec == EC - 1)
                for fc in range(EC):
                    nc.scalar.activation(
                        out=r1T[:, fc, :], in_=ps[:, fc * P : (fc + 1) * P],
                        func=ACTF.Relu,
                        bias=biases["ff1b"][:, fc : fc + 1],
                    )
                ps2 = ppb.tile([P, E], F32, tag="ppb")
                for fc in range(EC):
                    _mm(nc, ps2[:], r1T[:, fc, :], ff2w_t[:, fc, :],
                        fc == 0, fc == EC - 1)
                s2 = tmp_p.tile([P, E], F32, tag="s2")
                nc.vector.tensor_add(out=s2[:], in0=ps2[:], in1=biases["ff2b"][:])
                nc.gpsimd.tensor_add(out=s2[:], in0=s2[:], in1=acc[:, tcc, :])
                mv2, rs2 = ln_stats(s2[:], "ln2")
                xbf = ff_p.tile([P, E], MM_DT, tag="xbf")
                nc.vector.tensor_scalar(
                    out=xbf[:], in0=s2[:], scalar1=mv2[:, 0:1], scalar2=rs2[:],
                    op0=ALU.subtract, op1=ALU.mult,
                )
                nc.gpsimd.tensor_scalar(
                    out=xN_new[:, tcc, :], in0=s2[:], scalar1=mv2[:, 0:1],
                    scalar2=rs2[:], op0=ALU.subtract, op1=ALU.mult,
                )
                if dev_affine:
                    nc.vector.tensor_mul(out=xbf[:], in0=xbf[:], in1=biases["ln2w"][:])
                    nc.vector.tensor_add(out=xbf[:], in0=xbf[:], in1=biases["ln2b"][:])
                    nc.gpsimd.tensor_mul(
                        out=xN_new[:, tcc, :], in0=xN_new[:, tcc, :],
                        in1=biases["ln2w"][:])
                    nc.gpsimd.tensor_add(
                        out=xN_new[:, tcc, :], in0=xN_new[:, tcc, :],
                        in1=biases["ln2b"][:])
                nc.scalar.dma_start_transpose(out=xT_new[:, tcc, :, :], in_=xbf[:])
                nc.gpsimd.tensor_copy(xT8_new[:, tcc, :, :], xT_new[:, tcc, :, :])
            return chunkB

        if not last:
            acc = act_p.tile([P, TC, E], F32, tag="acc")
            xN_new = act_p.tile([P, TC, E], F32, tag="xN")
            xT_new = act_p.tile([P, TC, EC, P], MM_DT, tag="xT")
            xT8_new = act_p.tile([P, TC, EC, P], F8, tag="xT8")
            wts = []
            ffw = []
            h1Ts = {}
            for w in range(2):
                w2 = 2 * w
                prev = None
                for h in range(H):
                    if w == 0:
                        if (l, h) in preW:
                            wqk_t, wvo_t = preW[l, h]
                        else:
                            wqk_t = wtile(wq_p, wqk[l, h], "w8", dtype=F8)
                            wvo_t = wtile(wq_p, wvo[l, h], "w")
                        wts.append((wqk_t, wvo_t))
                        if h in (1, 2, 3):
                            load_biases(h)
                        if h == 5:
                            ffw.append((wtile(wff_p, ff1w[l], "wff1", bufs=1),
                                        wtile(wff_p, ff2w[l], "wff2", bufs=1)))
                    else:
                        wqk_t, wvo_t = wts[h]
                    # G^T = (x @ Wqk)^T for this wave's 256 tokens, via
                    # fp8 DoubleRow (2 rows/cycle); two oc column-groups
                    # share one PSUM bank and one evac copy, which also
                    # applies the 1/64 weight-prescale correction
                    GTt = qkv_p.tile([P, EC, T], MM_DT, tag="gt")
                    for oc2 in range(EC // 2):
                        ps = ppg.tile([P, 2 * T], F32, tag="ppg")
                        for k in range(2):
                            oc = 2 * oc2 + k
                            for j in range(2):
                                _c0 = k * T + j * P
                                for e2 in range(EC // 2):
                                    _mm8(nc, ps[:, _c0 : _c0 + P],
                                         wqk_t[:, 2 * e2 : 2 * e2 + 2,
                                               oc * P : (oc + 1) * P],
                                         xT8[:, w2 + j, 2 * e2 : 2 * e2 + 2, :],
                                         e2 == 0, e2 == EC // 2 - 1)
                        evac(GTt[:, 2 * oc2 : 2 * oc2 + 2, :], ps[:], oc2 % 2,
                             scale=1.0 / SW8)
                    # V' = x @ Wvo, token-major
                    Vp = qkv_p.tile([P, 2, E], MM_DT, tag="vp", bufs=3)
                    for j in range(2):
                        ps = ppb.tile([P, E], F32, tag="ppb")
                        for ec in range(EC):
                            _mm(nc, ps[:], xT[:, w2 + j, ec, :],
                                wvo_t[:, ec, :], ec == 0, ec == EC - 1)
                        evac(Vp[:, j, :], ps[:], j)
                    # energy, transposed [j, i]; den packed in the same bank
                    et = ppe.tile([P, 3 * P + 2], F32, tag="ppe")
                    e0 = et[:, 0:T]
                    e1 = et[:, T : 3 * P]
                    den = et[:, 3 * P : 3 * P + 2]
                    for ec in range(EC):
                        _mm(nc, e0, xT[:, w2, ec, :], GTt[:, ec, :],
                            ec == 0, ec == EC - 1)
                    for ec in range(EC):
                        _mm(nc, e1, xT[:, w2 + 1, ec, :], GTt[:, ec, P:T],
                            ec == 0, ec == EC - 1)
                    nc.vector.tensor_add(out=et[:, 0:P], in0=et[:, 0:P],
                                         in1=maskd_t[:])
                    nc.vector.tensor_add(out=e1, in0=e1, in1=maskd_t[:])
                    expT = esb_p.tile([P, 3 * P], MM_DT, tag="expT")
                    nc.scalar.activation(out=expT[:, 0:T], in_=e0, func=ACTF.Exp)
                    nc.scalar.activation(out=expT[:, T : T + P], in_=e1,
                                         func=ACTF.Exp)
                    if h == 1 and carry is not None:
                        # two heads of projections precede the carried tail:
                        # the new wave's first exp clears the Act queue before
                        # its own tail needs it
                        carry()
                        carry = None
                    if h >= 1:
                        tail(*prev)
                    prev = (h, expT, Vp, w2, acc, den)
                    if h in (3, 5) and pendingB:
                        pendingB.pop(0)()
                # wave end: defer tail(7) + part-A chunks into the next wave
                A0 = make_ffn_A(w2, acc, xN, h1Ts, biases)
                A1 = make_ffn_A(w2 + 1, acc, xN, h1Ts, biases)

                def carry(prev=prev, A0=A0, A1=A1):
                    tail(*prev)
                    A0()
                    A1()
                if w == 1 and l == L - 2 and nlayers == L and last_opt:
                    # the last layer's weight-DMA flood would delay these
                    # chunks' xbar transposes; emit them before it starts
                    carry()
                    carry = None
                pendingB.append(make_ffn_B(w2, acc, xN_new, xT_new,
                                           xT8_new, h1Ts, biases, ffw))
                pendingB.append(make_ffn_B(w2 + 1, acc, xN_new, xT_new,
                                           xT8_new, h1Ts, biases, ffw))
            xN = xN_new
            xT = xT_new
            xT8 = xT8_new
        else:
            # ---- last layer: attention only for the final token per batch ----
            if carry is not None:
                carry()
                carry = None
            ffw = []
            ao_b = [
                out_p.tile([1, E], F32, name=f"ao{b}", tag=f"ao{b}")
                for b in range(BPC)
            ]

            def last_pse(h, b, GTl):
                pse = ppg.tile([1, T], F32, tag="ppg")
                for jc in range(2):
                    for ec in range(EC):
                        _mm(nc, pse[:, jc * P : (jc + 1) * P],
                            GTl[:, ec : ec + 1], xT[:, 2 * b + jc, ec, :],
                            ec == 0, ec == EC - 1)
                att = esb_p.tile([1, T], F32, tag="esbl", bufs=5)
                nmax = st_p.tile([1, 1], F32, tag="nmaxl")
                nc.vector.reduce_max(out=nmax[:], in_=pse[:], axis=AX.X, negate=True)
                dn = st_p.tile([1, 1], F32, tag="denl")
                nc.scalar.activation(
                    out=att[:], in_=pse[:], func=ACTF.Exp,
                    bias=nmax[0:1, 0:1], accum_out=dn[:],
                )
                rec = st_p.tile([1, 1], F32, tag="recl")
                nc.vector.reciprocal(out=rec[:], in_=dn[:])
                nc.vector.tensor_scalar_mul(out=att[:], in0=att[:], scalar1=rec[:])
                return att

            def last_fin(h, b, att, Vp, Ul):
                attTl = esb_p.tile([P, 2, 1], MM_DT, tag="attTl", bufs=4)
                for jc in range(2):
                    tp = ppg.tile([P, P], F32, tag="ppg")
                    nc.tensor.transpose(
                        tp[:, 0:1], att[0:1, jc * P : (jc + 1) * P],
                        ident_t[0:1, 0:1]
                    )
                    evac(attTl[:, jc, 0:1], tp[:, 0:1], jc % 2)
                _mm(nc, Ul, attTl[:, 0, 0:1], Vp[:, 0, :], h == 0, False,
                    skip=True)
                _mm(nc, Ul, attTl[:, 1, 0:1], Vp[:, 1, :], False, h == H - 1,
                    skip=True)

            xlTs = {}

            def last_ffn(b, Ul):
                """FFN + LN for batch b's final token ([1, E] rows); emitted
                right after pass b so it overlaps the next pass's PE work."""
                ff1w_t, ff2w_t = ffw[0]
                ao = ao_b[b]
                nc.vector.scalar_tensor_tensor(
                    out=ao[:], in0=Ul, scalar=1.0, in1=biases["bo"][0:1, :],
                    op0=ALU.mult, op1=ALU.add,
                )
                x_lb = out_p.tile([1, E], F32, name=f"xl{b}", tag=f"xl{b}")
                nc.sync.dma_start(out=x_lb[:], in_=xN[P - 1 : P, 2 * b + 1, :])
                s1t = tmp_p.tile([P, E], F32, tag="s1")
                s1 = s1t[0:1]
                nc.vector.tensor_add(out=s1, in0=ao[:], in1=x_lb[:])
                h1t = tmp_p.tile([P, E], F32, tag="s2")
                h1_l = h1t[0:1]
                layernorm(s1, h1_l, None, None, "lnL1", rows=1)
                h1T_l = ff_p.tile([P, EC, 1], MM_DT, tag="h1Tl", bufs=2)
                for bb in range(EC):
                    tp = ppg.tile([P, P], F32, tag="ppg")
                    nc.tensor.transpose(
                        tp[:, 0:1], h1_l[:, bb * P : (bb + 1) * P],
                        ident_t[0:1, 0:1]
                    )
                    evac(h1T_l[:, bb, :], tp[:, 0:1], bb % 2)
                r1T_l = ff_p.tile([P, EC, 1], MM_DT, tag="r1Tl", bufs=2)
                ps = ppg.tile([P, P], F32, tag="ppg")
                for fc in range(EC):
                    for ec in range(EC):
                        _mm(nc, ps[:, fc : fc + 1],
                            ff1w_t[:, ec, fc * P : (fc + 1) * P],
                            h1T_l[:, ec, :], ec == 0, ec == EC - 1)
                for fc in range(EC):
                    nc.scalar.activation(
                        out=r1T_l[:, fc, :], in_=ps[:, fc : fc + 1],
                        func=ACTF.Relu,
                        bias=biases["ff1b"][:, fc : fc + 1],
                    )
                ps2 = ppb.tile([1, E], F32, tag="ppb")
                for fc in range(EC):
                    _mm(nc, ps2[:], r1T_l[:, fc, :], ff2w_t[:, fc, :],
                        fc == 0, fc == EC - 1)
                s2t = tmp_p.tile([P, E], F32, tag="s1")
                s2 = s2t[0:1]
                nc.vector.tensor_add(out=s2, in0=ps2[:], in1=biases["ff2b"][0:1, :])
                nc.vector.tensor_add(out=s2, in0=s2, in1=ao[:])
                xlt = tmp_p.tile([P, E], F32, tag="s2")
                xl = xlt[0:1]
                layernorm(s2, xl,
                          biases.get("ln2w") if dev_affine else None,
                          biases.get("ln2b") if dev_affine else None,
                          "lnL2", rows=1)
                xlT = ff_p.tile([P, EC, 1], MM_DT, tag="xlT", bufs=2)
                for bb in range(EC):
                    tp = ppg.tile([P, P], F32, tag="ppg")
                    nc.tensor.transpose(
                        tp[:, 0:1], xl[:, bb * P : (bb + 1) * P],
                        ident_t[0:1, 0:1]
                    )
                    evac(xlT[:, bb, :], tp[:, 0:1], bb % 2)
                xlTs[b] = xlT

            # two batch passes, each with a 2-deep softmax pipeline; pass 0
            # only touches token chunks 0/1, so the still-in-flight FFN
            # chunks for chunks 2/3 (pendingB) finish underneath it
            wts_l = []
            for bp in range(BPC):
                stage1 = []    # (h, GTl, Vp) awaiting pse
                stage2 = []    # (h, att, Vp) awaiting finish
                Ulp = ppe.tile([P, E], F32, tag="ppe", name=f"Ul{bp}")
                Ul = Ulp[0:1, 0:E]
                for h in range(H):
                    if bp == 0:
                        wqk_t = wtile(wq_p, wqk[l, h], "w8", dtype=F8)
                        wvo_t = wtile(wq_p, wvo[l, h], "w")
                        wts_l.append((wqk_t, wvo_t))
                        if h in (1, 2, 3):
                            load_biases(h)
                        if h == 5:
                            ffw.append((wtile(wff_p, ff1w[l], "wff1", bufs=1),
                                        wtile(wff_p, ff2w[l], "wff2", bufs=1)))
                    else:
                        wqk_t, wvo_t = wts_l[h]
                    Vp = qkv_p.tile([P, 2, E], MM_DT, tag="vpl", bufs=4)
                    for j in range(2):
                        ps = ppb.tile([P, E], F32, tag="ppb")
                        for ec in range(EC):
                            _mm(nc, ps[:], xT[:, 2 * bp + j, ec, :],
                                wvo_t[:, ec, :], ec == 0, ec == EC - 1)
                        evac(Vp[:, j, :], ps[:], j)
                    if bp == 0 and h in (2, 4) and pendingB:
                        pendingB.pop(0)()
                    GTl = qkv_p.tile([P, EC], MM_DT, tag="qtl", bufs=3)
                    ps = ppg.tile([P, T], F32, tag="ppg")
                    for oc in range(EC):
                        for e2 in range(EC // 2):
                            _mm8(nc, ps[:, oc : oc + 1],
                                 wqk_t[:, 2 * e2 : 2 * e2 + 2,
                                       oc * P : (oc + 1) * P],
                                 xT8[:, 2 * bp + 1, 2 * e2 : 2 * e2 + 2,
                                     P - 1 : P],
                                 e2 == 0, e2 == EC // 2 - 1)
                    evac(GTl[:], ps[:, 0:EC], 0, scale=1.0 / SW8)
                    stage1.append((h, GTl, Vp))
                    if len(stage1) > 1:
                        h1_, G1, V1 = stage1.pop(0)
                        stage2.append((h1_, last_pse(h1_, bp, G1), V1))
                    if len(stage2) > 1:
                        h2_, att2, V2 = stage2.pop(0)
                        last_fin(h2_, bp, att2, V2, Ul)
                while stage1:
                    h1_, G1, V1 = stage1.pop(0)
                    stage2.append((h1_, last_pse(h1_, bp, G1), V1))
                while stage2:
                    h2_, att2, V2 = stage2.pop(0)
                    last_fin(h2_, bp, att2, V2, Ul)
                last_ffn(bp, Ul)

    # ---- output head: last token of each batch ----
    if carry is not None:
        carry()
        carry = None
    while pendingB:   # nlayers < L debug builds leave the tail chunks
        pendingB.pop(0)()
    wout_t = wff_p.tile([P, EC, V], MM_DT, tag="wout", bufs=1)
    nc.sync.dma_start(
        out=wout_t[:], in_=wout.rearrange("(c p) o -> p c o", p=P)
    )
    bout_t = out_p.tile([1, V], F32)
    nc.sync.dma_start(out=bout_t[:], in_=bout.partition_broadcast(1))
    for b in range(BPC):
        pl = ppb.tile([1, V], F32, tag="ppb")
        if nlayers == L and last_opt:
            for ec in range(EC):
                _mm(nc, pl[:], xlTs[b][:, ec, :], wout_t[:, ec, :],
                    ec == 0, ec == EC - 1)
        else:
            for ec in range(EC):
                _mm(nc, pl[:], xT[:, 2 * b + 1, ec, P - 1 : P],
                    wout_t[:, ec, :], ec == 0, ec == EC - 1)
        logits = out_p.tile([1, V], F32, name=f"lg{b}", tag=f"lg{b}")
        nc.vector.tensor_add(out=logits[:], in0=pl[:], in1=bout_t[:])
        nmax = out_p.tile([1, 1], F32, name=f"nm{b}", tag=f"nm{b}")
        nc.vector.reduce_max(out=nmax[:], in_=logits[:], axis=AX.X, negate=True)
        den = out_p.tile([1, 1], F32, name=f"dn{b}", tag=f"dn{b}")
        nc.scalar.activation(
            out=logits[:], in_=logits[:], func=ACTF.Exp,
            bias=nmax[:, 0:1], accum_out=den[:],
        )
        rec = out_p.tile([1, 1], F32, name=f"rc{b}", tag=f"rc{b}")
        nc.vector.reciprocal(out=rec[:], in_=den[:])
        nc.vector.tensor_scalar_mul(out=logits[:], in0=logits[:], scalar1=rec[:])
        nc.sync.dma_start(out=probs[b : b + 1, :], in_=logits[:])


def _pe_table():
    i = np.arange(E, dtype=np.float32)
    rates = (1.0 / np.power(np.float32(10000.0), 2.0 * np.floor(i / 2.0) / E)).astype(
        np.float32
    )
    ang = np.arange(T, dtype=np.float32)[:, None] * rates[None, :]
    pe = np.concatenate([np.sin(ang[:, 0::2]), np.cos(ang[:, 1::2])], axis=-1)
    return np.tile(pe.astype(np.float32), (BPC, 1))  # [TOK, E]


def _maskd():
    j = np.arange(P)
    return np.where(j[None, :] < j[:, None], np.float32(NEG), np.float32(0.0))


def _prep_in_maps(
    input_tokens, emb, wq, wk, wv, wo, bo, ln1_w, ln1_b, ln2_w, ln2_b,
    ff1_w, ff1_b, ff2_w, ff2_b, wout, bout, fold_ln2=True,
):
    f = lambda x: np.ascontiguousarray(np.asarray(x, dtype=np.float32))
    w = lambda x: np.ascontiguousarray(np.asarray(x, dtype=np.float32).astype(NP_WDT))
    wq_ = np.asarray(wq, dtype=np.float32)
    wk_ = np.asarray(wk, dtype=np.float32)
    wv_ = np.asarray(wv, dtype=np.float32)
    wo_ = np.asarray(wo, dtype=np.float32).reshape(L, H, E, E)
    wqk = np.matmul(wq_, np.swapaxes(wk_, -1, -2))
    wvo = np.matmul(wv_, wo_)
    wout_ = np.asarray(wout, dtype=np.float32)
    ln2_w_ = np.asarray(ln2_w, dtype=np.float32)
    if fold_ln2:
        # x_{l+1} = LN2core(s2) * ln2_w (ln2_b == 0), so fold ln2_w[l] into
        # the next layer's Wqk (both sides) / Wvo (left side) and into wout.
        wqk = wqk.copy()
        wvo = wvo.copy()
        for l in range(1, L):
            s = ln2_w_[l - 1]
            wqk[l] = s[None, :, None] * wqk[l] * s[None, None, :]
            wvo[l] = s[None, :, None] * wvo[l]
        wout_ = ln2_w_[L - 1][:, None] * wout_
    # fold LN1's affine params into ff1 (exact):
    # relu((x*w1+b1) @ W + b) == relu(x @ (w1[:,None]*W) + (b1 @ W + b))
    ln1_w_ = np.asarray(ln1_w, dtype=np.float32)
    ln1_b_ = np.asarray(ln1_b, dtype=np.float32)
    ff1w_ = np.asarray(ff1_w, dtype=np.float32) * ln1_w_[:, :, None]
    ff1b_ = np.asarray(ff1_b, dtype=np.float32) + np.einsum(
        "le,leo->lo", ln1_b_, np.asarray(ff1_w, dtype=np.float32)
    )
    toks = np.asarray(input_tokens).astype(np.int64)
    wqk8 = np.ascontiguousarray((wqk * SW8).astype(NP_F8))
    shared = {
        "emb": f(emb), "wqk": wqk8, "wvo": w(wvo),
        "bo": f(bo), "ln2w": f(ln2_w), "ln2b": f(ln2_b),
        "ff1w": w(ff1w_), "ff1b": f(ff1b_), "ff2w": w(ff2_w),
        "ff2b": f(ff2_b), "wout": w(wout_), "bout": f(bout),
        "pe2": w(_pe_table()), "maskd": _maskd(),
        "ident": np.eye(P, dtype=np.float32),
    }
    in_maps = []
    for c in range(NCORES):
        t = toks[c * BPC : (c + 1) * BPC].reshape(TOK)  # [512] flat tokens
        tokarr = np.ascontiguousarray(t.reshape(TC, P).T.astype(np.int32))
        in_maps.append({**shared, "tok": tokarr})
    return in_maps


def kernel(**inputs):
    ln2_b = np.asarray(inputs["ln2_b"], dtype=np.float32)
    fold = not np.any(ln2_b != 0.0)
    key = "nc" if fold else "nc_affine"
    if key not in _CACHE:
        _CACHE[key] = _build(dev_affine=not fold)
    nc = _CACHE[key]
    in_maps = _prep_in_maps(**inputs, fold_ln2=fold)
    res = run_bass_kernel_spmd(nc, in_maps, core_ids=list(range(NCORES)))
    _CACHE["last_results"] = res
    out = np.concatenate([res.results[c]["probs"] for c in range(NCORES)], axis=0)
    return out.astype(np.float32)


# revision 64
# speedup vs baseline: 1.1099x; 1.1099x over previous
"""CheckersGPT dense transformer forward pass on 8 Trainium2 NeuronCores.

Strategy: pure data-parallel over the batch dim (16 batches -> 2 per core).
Each core runs the full 6-layer transformer on its 512 tokens (2 batches x
256 tokens); no collectives, outputs concatenated on the host.

Key restructuring vs a direct translation (each head uses a full ExE Q/K/V):
  - Host folds Wqk = Wq @ Wk^T and Wvo_h = Wv_h @ Wo_h. The K projection and
    the H*E->E output projection disappear: energy = (x Wqk) x^T and
    attn_out = sum_h att_h (x Wvo_h).
  - LN1's affine params are folded into ff1 (exact); LN2's scale is folded
    into the next layer's Wqk/Wvo/wout (exact when ln2_b == 0, which holds
    for this model); bo is folded into the h==0 attention accumulation.
  - The two batches per core are INDEPENDENT sequences, so each layer's
    attention is split into two "waves" (batch 0, batch 1). The FFN +
    layernorm chains of wave b run on DVE/Act/Pool underneath the ~38us of
    PE projection work of the opposite wave, so the PE never idles at layer
    boundaries. Each FFN token-chunk is emitted as a PE-free part A
    (residual + LN1 + transposed-h1) right after its wave's last tail, and
    a part B (ff1/ff2/LN2/new-x) two heads into the opposite wave.
  - Energy is computed TRANSPOSED ([j, i] = key-major) with xT as lhsT, so
    att @ V produces attn_out in natural token-major layout directly. Softmax
    runs without max-subtraction (energies are bounded by the 0.02-scale
    weights); denominators come from N=1 matmuls against a ones vector, and
    1/den is applied by a fused (U * rec) + acc scalar_tensor_tensor.
  - All natural->transposed layout changes for matmul feeds go through the
    DMA xbar transpose engine (dma_start_transpose, bf16), not the PE.
  - 1/sqrt(var+eps) is computed as exp(-0.5*ln(var+eps)): ln/exp/relu/copy
    live in one activation table set, so the Act engine never swaps tables.

Layout per core (P=128 partitions):
  xT   [128, TC, EC, 128] : x transposed; [p, tcc, ec, t] = token chunk tcc,
                            embed dim ec*128+p, token t. matmul lhsT/rhs.
  xN   [128, TC, E]       : x natural; residuals / layernorm.
All matmuls are out = lhsT.T @ rhs with contraction on the partition dim.
The last layer only computes attention/FFN for the final token of each batch.
"""

import os
import numpy as np
from contextlib import ExitStack

import ml_dtypes
import concourse.bass as bass
import concourse.tile as tile
from concourse import bacc, mybir
from concourse.bass_utils import run_bass_kernel_spmd

F32 = mybir.dt.float32
BF16 = mybir.dt.bfloat16
I32 = mybir.dt.int32
AX = mybir.AxisListType
ALU = mybir.AluOpType
ACTF = mybir.ActivationFunctionType

V, E, L, H, B, T = 512, 512, 6, 8, 16, 256
NCORES = 8
BPC = B // NCORES          # batches per core
TOK = BPC * T              # tokens per core
P = 128
EC = E // P                # embed chunks of 128
TC = TOK // P              # token chunks of 128
NEG = -1e9
EPS = 1e-5

MM_DT = BF16
F8 = mybir.dt.float8e4
NP_WDT = ml_dtypes.bfloat16
NP_F8 = ml_dtypes.float8_e4m3
SW8 = 64.0                 # host pre-scale on the fp8 Wqk weights
W_BUFS = 11                # weight tiles in flight per kind (1 per head each)

_CACHE = {}


def _mm(nc, out, lhsT, rhs, start, stop, skip=False):
    nc.tensor.matmul(out, lhsT, rhs, start=start, stop=stop,
                     skip_group_check=skip)


def _mm8(nc, out, lhsT, rhs, start, stop):
    nc.tensor.matmul(out, lhsT, rhs, start=start, stop=stop,
                     perf_mode=mybir.MatmulPerfMode.DoubleRow)


def _build(nlayers=L, reps=1, last_opt=True, dev_affine=False):
    nc = bacc.Bacc("TRN2", target_bir_lowering=False, debug=False, num_devices=NCORES)

    def din(name, shape, dtype=F32):
        return nc.dram_tensor(name, list(shape), dtype, kind="ExternalInput").ap()

    tok = din("tok", [P, TC], I32)            # token ids, p-major within chunks
    emb = din("emb", [V, E])
    pe2 = din("pe2", [TOK, E], MM_DT)         # positional encoding tiled over BPC
    wqk = din("wqk", [L, H, E, E], MM_DT)     # Wq @ Wk^T   (ln2 scale folded)
    wvo = din("wvo", [L, H, E, E], MM_DT)     # Wv @ Wo_h   (ln2 scale folded)
    bo = din("bo", [L, E])
    ln2w = din("ln2w", [L, E])
    ln2b = din("ln2b", [L, E])
    ff1w = din("ff1w", [L, E, E], MM_DT)      # ln1_w/ln1_b pre-folded on host
    ff1b = din("ff1b", [L, E])
    ff2w = din("ff2w", [L, E, E], MM_DT)
    ff2b = din("ff2b", [L, E])
    wout = din("wout", [E, V], MM_DT)         # ln2 scale of last layer folded
    bout = din("bout", [V])
    maskd = din("maskd", [P, P])              # additive causal mask, diag block
    ident = din("ident", [P, P])
    probs = nc.dram_tensor("probs", [BPC, V], F32, kind="ExternalOutput").ap()
    aps = (emb, pe2, wqk, wvo, bo, ln2w, ln2b,
           ff1w, ff1b, ff2w, ff2b, wout, bout, maskd, ident, probs, tok)

    with tile.TileContext(nc) as tc, ExitStack() as ctx:
        if reps > 1:
            with tc.For_i(0, reps, 1):
                _emit(nc, tc, ctx, aps, nlayers, last_opt, dev_affine)
        else:
            _emit(nc, tc, ctx, aps, nlayers, last_opt, dev_affine)

    nc.compile()
    return nc


def _emit(nc, tc, ctx, aps, nlayers, last_opt, dev_affine):
    (emb, pe2, wqk, wvo, bo, ln2w, ln2b,
     ff1w, ff1b, ff2w, ff2b, wout, bout, maskd, ident, probs, tok) = aps
    ep = ctx.enter_context

    const = ep(tc.tile_pool(name="const", bufs=1))
    wq_p = ep(tc.tile_pool(name="wq", bufs=W_BUFS))
    wff_p = ep(tc.tile_pool(name="wff", bufs=2))
    bias_p = ep(tc.tile_pool(name="bias", bufs=2))
    act_p = ep(tc.tile_pool(name="act", bufs=2))
    qkv_p = ep(tc.tile_pool(name="qkvact", bufs=2))
    ff_p = ep(tc.tile_pool(name="ffact", bufs=2))
    tmp_p = ep(tc.tile_pool(name="tmp", bufs=2))
    esb_p = ep(tc.tile_pool(name="esb", bufs=3))
    st_p = ep(tc.tile_pool(name="stats", bufs=4))
    out_p = ep(tc.tile_pool(name="outp", bufs=1))

    ppb = ep(tc.tile_pool(name="ppb", bufs=3, space="PSUM"))
    ppg = ep(tc.tile_pool(name="ppg", bufs=3, space="PSUM"))
    ppe = ep(tc.tile_pool(name="ppe", bufs=2, space="PSUM"))

    def wtile(pool, dram2d, tag, bufs=None, dtype=MM_DT):
        t = pool.tile([P, EC, E], dtype, tag=tag, bufs=bufs)
        nc.sync.dma_start(
            out=t[:], in_=dram2d.rearrange("(c p) o -> p c o", p=P)
        )
        return t

    def bbcast(vec_ap, tag):
        t = bias_p.tile([P, E], F32, tag=tag)
        nc.sync.dma_start(out=t[:], in_=vec_ap.partition_broadcast(P))
        return t

    # ---- constants; tok first (the embedding gather gates the whole start),
    # then layer-0 head-0 weights ahead of the bulk constants so the first
    # GT matmuls are not starved behind the 1MB positional-encoding DMA ----
    tok_t = const.tile([P, TC], I32)
    nc.sync.dma_start(out=tok_t[:], in_=tok)
    eps_t = const.tile([P, 1], F32)
    nc.vector.memset(eps_t[:], EPS)
    ones_t = const.tile([P, 1], MM_DT)
    nc.vector.memset(ones_t[:], 1.0)
    # startup order on the SP queue: wqk00, pe2-chunk0, wvo00, pe2 rest,
    # maskd, ident — so the first gather chunk's x tile and the first head's
    # weights are both ready ~5us in. The Act hwdge queue stays empty so the
    # init xbar transposes issue with no backlog.
    wqk00 = wtile(wq_p, wqk[0, 0], "w8", bufs=9)
    pe_t = const.tile([P, TC, E], MM_DT)
    pe2r = pe2.rearrange("(c p) o -> p c o", p=P)
    nc.sync.dma_start(out=pe_t[:, 0, :], in_=pe2r[:, 0, :])
    preW = {(0, 0): (wqk00, wtile(wq_p, wvo[0, 0], "w", bufs=9))}
    for c in range(1, TC):
        nc.sync.dma_start(out=pe_t[:, c, :], in_=pe2r[:, c, :])
    maskd_t = const.tile([P, P], F32)
    nc.sync.dma_start(out=maskd_t[:], in_=maskd)
    ident_t = const.tile([P, P], F32)
    nc.sync.dma_start(out=ident_t[:], in_=ident)

    def evac(dst, src, use_act, scale=None):
        """PSUM -> SBUF copy (dtype conversion happens on write)."""
        if use_act:
            if scale is None:
                nc.scalar.copy(dst, src)
            else:
                nc.scalar.activation(out=dst, in_=src, func=ACTF.Copy,
                                     scale=scale)
        else:
            if scale is None:
                nc.vector.tensor_copy(dst, src)
            else:
                nc.vector.tensor_scalar(out=dst, in0=src, scalar1=scale,
                                        scalar2=None, op0=ALU.mult,
                                        op1=ALU.bypass)

    def ln_stats(src, tag, rows=P):
        """mean + 1/sqrt(var+eps) of src [rows, E].

        rsqrt via the bit-trick + 2 Newton steps, entirely on DVE: keeps
        sqrt/ln off the Act engine so it never reloads its function table
        (exp/relu/copy all live in the one 'exp_and_others' set).
        """
        stt = st_p.tile([P, 6], F32, tag=tag + "s")
        nc.vector.bn_stats(out=stt[:rows], in_=src)
        mv = st_p.tile([P, 2], F32, tag=tag + "m")
        nc.vector.bn_aggr(out=mv[:rows], in_=stt[:rows])
        vp = st_p.tile([P, 1], F32, tag=tag + "v")
        nc.vector.tensor_scalar(
            out=vp[:rows], in0=mv[:rows, 1:2], scalar1=EPS, scalar2=None,
            op0=ALU.add, op1=ALU.bypass,
        )
        hh = st_p.tile([P, 1], F32, tag=tag + "h")
        nc.vector.tensor_scalar(
            out=hh[:rows], in0=vp[:rows], scalar1=-0.5, scalar2=None,
            op0=ALU.mult, op1=ALU.bypass,
        )
        rs = st_p.tile([P, 1], F32, tag=tag + "r")
        nc.vector.tensor_scalar(
            out=rs[:rows].bitcast(I32), in0=vp[:rows].bitcast(I32),
            scalar1=1, scalar2=None,
            op0=ALU.logical_shift_right, op1=ALU.bypass,
        )
        nc.vector.tensor_scalar(
            out=rs[:rows].bitcast(I32), in0=rs[:rows].bitcast(I32),
            scalar1=-1, scalar2=0x5F3759DF, op0=ALU.mult, op1=ALU.add,
        )
        for it in range(2):
            t0 = st_p.tile([P, 1], F32, tag=tag + "n", name=f"nw{tag}{it}")
            nc.vector.tensor_mul(out=t0[:rows], in0=rs[:rows], in1=rs[:rows])
            nc.vector.tensor_scalar(
                out=t0[:rows], in0=t0[:rows], scalar1=hh[:rows, 0:1],
                scalar2=1.5, op0=ALU.mult, op1=ALU.add,
            )
            nc.vector.tensor_mul(out=rs[:rows], in0=rs[:rows], in1=t0[:rows])
        return mv, rs

    def layernorm(src, dst, w_b, b_b, tag, rows=P):
        mv, rs = ln_stats(src, tag, rows)
        nc.vector.tensor_scalar(
            out=dst, in0=src, scalar1=mv[:rows, 0:1], scalar2=rs[:rows],
            op0=ALU.subtract, op1=ALU.mult,
        )
        if w_b is not None:
            nc.gpsimd.tensor_mul(out=dst, in0=dst, in1=w_b[:rows, :])
            nc.gpsimd.tensor_add(out=dst, in0=dst, in1=b_b[:rows, :])

    # ---- embedding gather + positional encoding ----
    xN = act_p.tile([P, TC, E], F32, tag="xN")
    for c in range(TC):
        nc.gpsimd.indirect_dma_start(
            out=xN[:, c, :], out_offset=None, in_=emb,
            in_offset=bass.IndirectOffsetOnAxis(ap=tok_t[:, c : c + 1], axis=0),
        )
    xT = act_p.tile([P, TC, EC, P], MM_DT, tag="xT")
    for c in range(TC):
        nc.vector.tensor_add(out=xN[:, c, :], in0=xN[:, c, :], in1=pe_t[:, c, :])
        xbf0 = ff_p.tile([P, E], MM_DT, tag="xbf")
        # DVE, not Pool: this copy sits on the startup-critical chain from
        # the embedding gather to the first projection matmuls
        nc.vector.tensor_copy(xbf0[:], xN[:, c, :])
        nc.scalar.dma_start_transpose(out=xT[:, c, :, :], in_=xbf0[:])

    carry = None       # tail(7) of the previous wave + its FFN part-A chunks
    pendingB = []      # FFN part-B chunks of the previous wave

    for l in range(nlayers):
        last = last_opt and (l == L - 1) and (nlayers == L)
        biases = {}

        def load_biases(stage, l=l):
            # staggered so bias DMAs don't jam the queue ahead of head weights
            if stage == 1:
                biases["bo"] = bbcast(bo[l], "b_bo")
            elif stage == 2:
                t = bias_p.tile([P, EC], F32, tag="b_f1")
                nc.sync.dma_start(
                    out=t[:], in_=ff1b[l].rearrange("(c p) -> p c", p=P)
                )
                biases["ff1b"] = t
            elif stage == 3:
                biases["ff2b"] = bbcast(ff2b[l], "b_f2")
                if dev_affine:
                    biases["ln2w"] = bbcast(ln2w[l], "b_l2w")
                    biases["ln2b"] = bbcast(ln2b[l], "b_l2b")

        def tail(h, expT, Vp, w2, acc, den, biases=biases):
            """den + AV + normalized accumulation for one wave-head.

            The (U * 1/den) + acc update alternates between a fused DVE
            scalar_tensor_tensor and an Act scaled-copy + Pool add, so no
            single vector engine eats the whole attention tail."""
            _mm(nc, den[:, 0:1], expT[:, 0:P], ones_t[:, 0:1], True, True)
            U0 = ppb.tile([P, E], F32, tag="ppb")
            _mm(nc, U0[:], expT[:, 0:P], Vp[:, 0, :], True, True)
            _mm(nc, den[:, 1:2], expT[:, P : 2 * P], ones_t[:, 0:1], True, False)
            U1 = ppb.tile([P, E], F32, tag="ppb")
            _mm(nc, U1[:], expT[:, P : 2 * P], Vp[:, 0, :], True, False)
            _mm(nc, den[:, 1:2], expT[:, 2 * P : 3 * P], ones_t[:, 0:1], False, True)
            _mm(nc, U1[:], expT[:, 2 * P : 3 * P], Vp[:, 1, :], False, True)
            rec = st_p.tile([P, 2], F32, tag="rec", bufs=3)
            nc.vector.reciprocal(out=rec[:], in_=den[:])
            for ic, U in ((0, U0), (1, U1)):
                tcc = w2 + ic
                prev_ap = biases["bo"][:] if h == 0 else acc[:, tcc, :]
                if ic == 0:
                    nc.vector.scalar_tensor_tensor(
                        out=acc[:, tcc, :], in0=U[:], scalar=rec[:, ic : ic + 1],
                        in1=prev_ap, op0=ALU.mult, op1=ALU.add,
                    )
                else:
                    un = tmp_p.tile([P, E], F32, tag="un", bufs=3)
                    nc.scalar.activation(
                        out=un[:], in_=U[:], func=ACTF.Copy,
                        scale=rec[:, ic : ic + 1],
                    )
                    if h == 0:
                        nc.gpsimd.tensor_add(out=acc[:, tcc, :], in0=un[:],
                                             in1=prev_ap)
                    else:
                        nc.gpsimd.tensor_add(out=acc[:, tcc, :],
                                             in0=acc[:, tcc, :], in1=un[:])

        def make_ffn_A(tcc, acc, xN, h1Ts, biases):
            def chunkA():
                s1 = tmp_p.tile([P, E], F32, tag="s1")
                nc.vector.tensor_add(
                    out=s1[:], in0=acc[:, tcc, :], in1=xN[:, tcc, :]
                )
                mv, rs = ln_stats(s1[:], "ln1")
                h1bf = ff_p.tile([P, E], MM_DT, tag="h1bf")
                nc.vector.tensor_scalar(
                    out=h1bf[:], in0=s1[:], scalar1=mv[:, 0:1], scalar2=rs[:],
                    op0=ALU.subtract, op1=ALU.mult,
                )
                h1T = ff_p.tile([P, EC, P], MM_DT, tag="h1T")
                nc.scalar.dma_start_transpose(out=h1T[:], in_=h1bf[:])
                h1Ts[tcc] = h1T
            return chunkA

        def make_ffn_B(tcc, acc, xN_new, xT_new, h1Ts, biases, ffw):
            def chunkB():
                ff1w_t, ff2w_t = ffw[0]
                h1T = h1Ts[tcc]
                r1T = ff_p.tile([P, EC, P], MM_DT, tag="r1T")
                ps = ppg.tile([P, EC * P], F32, tag="ppg")
                for fc in range(EC):
                    for ec in range(EC):
                        _mm(nc, ps[:, fc * P : (fc + 1) * P],
                            ff1w_t[:, ec, fc * P : (fc + 1) * P],
                            h1T[:, ec, :], ec == 0, ec == EC - 1)
                for fc in range(EC):
                    nc.scalar.activation(
                        out=r1T[:, fc, :], in_=ps[:, fc * P : (fc + 1) * P],
                        func=ACTF.Relu,
                        bias=biases["ff1b"][:, fc : fc + 1],
                    )
                ps2 = ppb.tile([P, E], F32, tag="ppb")
                for fc in range(EC):
                    _mm(nc, ps2[:], r1T[:, fc, :], ff2w_t[:, fc, :],
                        fc == 0, fc == EC - 1)
                s2 = tmp_p.tile([P, E], F32, tag="s2")
                nc.vector.tensor_add(out=s2[:], in0=ps2[:], in1=biases["ff2b"][:])
                nc.gpsimd.tensor_add(out=s2[:], in0=s2[:], in1=acc[:, tcc, :])
                mv2, rs2 = ln_stats(s2[:], "ln2")
                xbf = ff_p.tile([P, E], MM_DT, tag="xbf")
                nc.vector.tensor_scalar(
                    out=xbf[:], in0=s2[:], scalar1=mv2[:, 0:1], scalar2=rs2[:],
                    op0=ALU.subtract, op1=ALU.mult,
                )
                nc.gpsimd.tensor_scalar(
                    out=xN_new[:, tcc, :], in0=s2[:], scalar1=mv2[:, 0:1],
                    scalar2=rs2[:], op0=ALU.subtract, op1=ALU.mult,
                )
                if dev_affine:
                    nc.vector.tensor_mul(out=xbf[:], in0=xbf[:], in1=biases["ln2w"][:])
                    nc.vector.tensor_add(out=xbf[:], in0=xbf[:], in1=biases["ln2b"][:])
                    nc.gpsimd.tensor_mul(
                        out=xN_new[:, tcc, :], in0=xN_new[:, tcc, :],
                        in1=biases["ln2w"][:])
                    nc.gpsimd.tensor_add(
                        out=xN_new[:, tcc, :], in0=xN_new[:, tcc, :],
                        in1=biases["ln2b"][:])
                nc.scalar.dma_start_transpose(out=xT_new[:, tcc, :, :], in_=xbf[:])
            return chunkB

        if not last:
            acc = act_p.tile([P, TC, E], F32, tag="acc")
            xN_new = act_p.tile([P, TC, E], F32, tag="xN")
            xT_new = act_p.tile([P, TC, EC, P], MM_DT, tag="xT")
            wts = []
            ffw = []
            h1Ts = {}
            for w in range(2):
                w2 = 2 * w
                prev = None
                for h in range(H):
                    if w == 0:
                        if (l, h) in preW:
                            wqk_t, wvo_t = preW[l, h]
                        else:
                            wqk_t = wtile(wq_p, wqk[l, h], "w8", bufs=9)
                            wvo_t = wtile(wq_p, wvo[l, h], "w", bufs=9)
                        wts.append((wqk_t, wvo_t))
                        if h in (1, 2, 3):
                            load_biases(h)
                        if h == 5:
                            ffw.append((wtile(wff_p, ff1w[l], "wff1", bufs=1),
                                        wtile(wff_p, ff2w[l], "wff2", bufs=1)))
                    else:
                        wqk_t, wvo_t = wts[h]
                    # G^T = (x @ Wqk)^T for this wave's 256 tokens; two oc
                    # column-groups share one PSUM bank and one evac copy
                    GTt = qkv_p.tile([P, EC, T], MM_DT, tag="gt")
                    for oc2 in range(EC // 2):
                        ps = ppg.tile([P, 2 * T], F32, tag="ppg")
                        for k in range(2):
                            oc = 2 * oc2 + k
                            for j in range(2):
                                _c0 = k * T + j * P
                                for ec in range(EC):
                                    _mm(nc, ps[:, _c0 : _c0 + P],
                                        wqk_t[:, ec, oc * P : (oc + 1) * P],
                                        xT[:, w2 + j, ec, :],
                                        ec == 0, ec == EC - 1)
                        evac(GTt[:, 2 * oc2 : 2 * oc2 + 2, :], ps[:], oc2 % 2)
                    # V' = x @ Wvo, token-major
                    Vp = qkv_p.tile([P, 2, E], MM_DT, tag="vp", bufs=3)
                    for j in range(2):
                        ps = ppb.tile([P, E], F32, tag="ppb")
                        for ec in range(EC):
                            _mm(nc, ps[:], xT[:, w2 + j, ec, :],
                                wvo_t[:, ec, :], ec == 0, ec == EC - 1)
                        evac(Vp[:, j, :], ps[:], j)
                    # energy, transposed [j, i]; den packed in the same bank
                    et = ppe.tile([P, 3 * P + 2], F32, tag="ppe")
                    e0 = et[:, 0:T]
                    e1 = et[:, T : 3 * P]
                    den = et[:, 3 * P : 3 * P + 2]
                    for ec in range(EC):
                        _mm(nc, e0, xT[:, w2, ec, :], GTt[:, ec, :],
                            ec == 0, ec == EC - 1)
                    for ec in range(EC):
                        _mm(nc, e1, xT[:, w2 + 1, ec, :], GTt[:, ec, P:T],
                            ec == 0, ec == EC - 1)
                    nc.vector.tensor_add(out=et[:, 0:P], in0=et[:, 0:P],
                                         in1=maskd_t[:])
                    nc.vector.tensor_add(out=e1, in0=e1, in1=maskd_t[:])
                    expT = esb_p.tile([P, 3 * P], MM_DT, tag="expT")
                    nc.scalar.activation(out=expT[:, 0:T], in_=e0, func=ACTF.Exp)
                    nc.scalar.activation(out=expT[:, T : T + P], in_=e1,
                                         func=ACTF.Exp)
                    if h == 1 and carry is not None:
                        # two heads of projections precede the carried tail:
                        # the new wave's first exp clears the Act queue before
                        # its own tail needs it
                        carry()
                        carry = None
                    if h >= 1:
                        tail(*prev)
                    prev = (h, expT, Vp, w2, acc, den)
                    if h in (3, 5) and pendingB:
                        pendingB.pop(0)()
                # wave end: defer tail(7) + part-A chunks into the next wave
                A0 = make_ffn_A(w2, acc, xN, h1Ts, biases)
                A1 = make_ffn_A(w2 + 1, acc, xN, h1Ts, biases)

                def carry(prev=prev, A0=A0, A1=A1):
                    tail(*prev)
                    A0()
                    A1()
                if w == 1 and l == L - 2 and nlayers == L and last_opt:
                    # the last layer's weight-DMA flood would delay these
                    # chunks' xbar transposes; emit them before it starts
                    carry()
                    carry = None
                pendingB.append(make_ffn_B(w2, acc, xN_new, xT_new,
                                           h1Ts, biases, ffw))
                pendingB.append(make_ffn_B(w2 + 1, acc, xN_new, xT_new,
                                           h1Ts, biases, ffw))
            xN = xN_new
            xT = xT_new
        else:
            # ---- last layer: attention only for the final token per batch ----
            if carry is not None:
                carry()
                carry = None
            ffw = []
            ao_b = [
                out_p.tile([1, E], F32, name=f"ao{b}", tag=f"ao{b}")
                for b in range(BPC)
            ]

            def last_pse(h, b, GTl):
                pse = ppg.tile([1, T], F32, tag="ppg")
                for jc in range(2):
                    for ec in range(EC):
                        _mm(nc, pse[:, jc * P : (jc + 1) * P],
                            GTl[:, ec : ec + 1], xT[:, 2 * b + jc, ec, :],
                            ec == 0, ec == EC - 1)
                att = esb_p.tile([1, T], F32, tag="esbl", bufs=5)
                nmax = st_p.tile([1, 1], F32, tag="nmaxl")
                nc.vector.reduce_max(out=nmax[:], in_=pse[:], axis=AX.X, negate=True)
                dn = st_p.tile([1, 1], F32, tag="denl")
                nc.scalar.activation(
                    out=att[:], in_=pse[:], func=ACTF.Exp,
                    bias=nmax[0:1, 0:1], accum_out=dn[:],
                )
                rec = st_p.tile([1, 1], F32, tag="recl")
                nc.vector.reciprocal(out=rec[:], in_=dn[:])
                nc.vector.tensor_scalar_mul(out=att[:], in0=att[:], scalar1=rec[:])
                return att

            def last_fin(h, b, att, Vp, Ul):
                attTl = esb_p.tile([P, 2, 1], MM_DT, tag="attTl", bufs=4)
                for jc in range(2):
                    tp = ppg.tile([P, P], F32, tag="ppg")
                    nc.tensor.transpose(
                        tp[:, 0:1], att[0:1, jc * P : (jc + 1) * P],
                        ident_t[0:1, 0:1]
                    )
                    evac(attTl[:, jc, 0:1], tp[:, 0:1], jc % 2)
                _mm(nc, Ul, attTl[:, 0, 0:1], Vp[:, 0, :], h == 0, False,
                    skip=True)
                _mm(nc, Ul, attTl[:, 1, 0:1], Vp[:, 1, :], False, h == H - 1,
                    skip=True)

            xlTs = {}

            def last_ffn(b, Ul):
                """FFN + LN for batch b's final token ([1, E] rows); emitted
                right after pass b so it overlaps the next pass's PE work."""
                ff1w_t, ff2w_t = ffw[0]
                ao = ao_b[b]
                nc.vector.scalar_tensor_tensor(
                    out=ao[:], in0=Ul, scalar=1.0, in1=biases["bo"][0:1, :],
                    op0=ALU.mult, op1=ALU.add,
                )
                x_lb = out_p.tile([1, E], F32, name=f"xl{b}", tag=f"xl{b}")
                nc.sync.dma_start(out=x_lb[:], in_=xN[P - 1 : P, 2 * b + 1, :])
                s1t = tmp_p.tile([P, E], F32, tag="s1")
                s1 = s1t[0:1]
                nc.vector.tensor_add(out=s1, in0=ao[:], in1=x_lb[:])
                h1t = tmp_p.tile([P, E], F32, tag="s2")
                h1_l = h1t[0:1]
                layernorm(s1, h1_l, None, None, "lnL1", rows=1)
                h1T_l = ff_p.tile([P, EC, 1], MM_DT, tag="h1Tl", bufs=2)
                for bb in range(EC):
                    tp = ppg.tile([P, P], F32, tag="ppg")
                    nc.tensor.transpose(
                        tp[:, 0:1], h1_l[:, bb * P : (bb + 1) * P],
                        ident_t[0:1, 0:1]
                    )
                    evac(h1T_l[:, bb, :], tp[:, 0:1], bb % 2)
                r1T_l = ff_p.tile([P, EC, 1], MM_DT, tag="r1Tl", bufs=2)
                ps = ppg.tile([P, P], F32, tag="ppg")
                for fc in range(EC):
                    for ec in range(EC):
                        _mm(nc, ps[:, fc : fc + 1],
                            ff1w_t[:, ec, fc * P : (fc + 1) * P],
                            h1T_l[:, ec, :], ec == 0, ec == EC - 1)
                for fc in range(EC):
                    nc.scalar.activation(
                        out=r1T_l[:, fc, :], in_=ps[:, fc : fc + 1],
                        func=ACTF.Relu,
                        bias=biases["ff1b"][:, fc : fc + 1],
                    )
                ps2 = ppb.tile([1, E], F32, tag="ppb")
                for fc in range(EC):
                    _mm(nc, ps2[:], r1T_l[:, fc, :], ff2w_t[:, fc, :],
                        fc == 0, fc == EC - 1)
                s2t = tmp_p.tile([P, E], F32, tag="s1")
                s2 = s2t[0:1]
                nc.vector.tensor_add(out=s2, in0=ps2[:], in1=biases["ff2b"][0:1, :])
                nc.vector.tensor_add(out=s2, in0=s2, in1=ao[:])
                xlt = tmp_p.tile([P, E], F32, tag="s2")
                xl = xlt[0:1]
                layernorm(s2, xl,
                          biases.get("ln2w") if dev_affine else None,
                          biases.get("ln2b") if dev_affine else None,
                          "lnL2", rows=1)
                xlT = ff_p.tile([P, EC, 1], MM_DT, tag="xlT", bufs=2)
                for bb in range(EC):
                    tp = ppg.tile([P, P], F32, tag="ppg")
                    nc.tensor.transpose(
                        tp[:, 0:1], xl[:, bb * P : (bb + 1) * P],
                        ident_t[0:1, 0:1]
                    )
                    evac(xlT[:, bb, :], tp[:, 0:1], bb % 2)
                xlTs[b] = xlT

            # two batch passes, each with a 2-deep softmax pipeline; pass 0
            # only touches token chunks 0/1, so the still-in-flight FFN
            # chunks for chunks 2/3 (pendingB) finish underneath it
            wts_l = []
            for bp in range(BPC):
                stage1 = []    # (h, GTl, Vp) awaiting pse
                stage2 = []    # (h, att, Vp) awaiting finish
                Ulp = ppe.tile([P, E], F32, tag="ppe", name=f"Ul{bp}")
                Ul = Ulp[0:1, 0:E]
                for h in range(H):
                    if bp == 0:
                        wqk_t = wtile(wq_p, wqk[l, h], "w8", bufs=9)
                        wvo_t = wtile(wq_p, wvo[l, h], "w", bufs=9)
                        wts_l.append((wqk_t, wvo_t))
                        if h in (1, 2, 3):
                            load_biases(h)
                        if h == 5:
                            ffw.append((wtile(wff_p, ff1w[l], "wff1", bufs=1),
                                        wtile(wff_p, ff2w[l], "wff2", bufs=1)))
                    else:
                        wqk_t, wvo_t = wts_l[h]
                    Vp = qkv_p.tile([P, 2, E], MM_DT, tag="vpl", bufs=4)
                    for j in range(2):
                        ps = ppb.tile([P, E], F32, tag="ppb")
                        for ec in range(EC):
                            _mm(nc, ps[:], xT[:, 2 * bp + j, ec, :],
                                wvo_t[:, ec, :], ec == 0, ec == EC - 1)
                        evac(Vp[:, j, :], ps[:], j)
                    if bp == 0 and h in (2, 4) and pendingB:
                        pendingB.pop(0)()
                    GTl = qkv_p.tile([P, EC], MM_DT, tag="qtl", bufs=3)
                    ps = ppg.tile([P, T], F32, tag="ppg")
                    for oc in range(EC):
                        for ec in range(EC):
                            _mm(nc, ps[:, oc : oc + 1],
                                wqk_t[:, ec, oc * P : (oc + 1) * P],
                                xT[:, 2 * bp + 1, ec, P - 1 : P],
                                ec == 0, ec == EC - 1)
                    evac(GTl[:], ps[:, 0:EC], 0)
                    stage1.append((h, GTl, Vp))
                    if len(stage1) > 1:
                        h1_, G1, V1 = stage1.pop(0)
                        stage2.append((h1_, last_pse(h1_, bp, G1), V1))
                    if len(stage2) > 1:
                        h2_, att2, V2 = stage2.pop(0)
                        last_fin(h2_, bp, att2, V2, Ul)
                while stage1:
                    h1_, G1, V1 = stage1.pop(0)
                    stage2.append((h1_, last_pse(h1_, bp, G1), V1))
                while stage2:
                    h2_, att2, V2 = stage2.pop(0)
                    last_fin(h2_, bp, att2, V2, Ul)
                last_ffn(bp, Ul)

    # ---- output head: last token of each batch ----
    if carry is not None:
        carry()
        carry = None
    while pendingB:   # nlayers < L debug builds leave the tail chunks
        pendingB.pop(0)()
    wout_t = wff_p.tile([P, EC, V], MM_DT, tag="wout", bufs=1)
    nc.sync.dma_start(
        out=wout_t[:], in_=wout.rearrange("(c p) o -> p c o", p=P)
    )
    bout_t = out_p.tile([1, V], F32)
    nc.sync.dma_start(out=bout_t[:], in_=bout.partition_broadcast(1))
    for b in range(BPC):
        pl = ppb.tile([1, V], F32, tag="ppb")
        if nlayers == L and last_opt:
            for ec in range(EC):
                _mm(nc, pl[:], xlTs[b][:, ec, :], wout_t[:, ec, :],
                    ec == 0, ec == EC - 1)
        else:
            for ec in range(EC):
                _mm(nc, pl[:], xT[:, 2 * b + 1, ec, P - 1 : P],
                    wout_t[:, ec, :], ec == 0, ec == EC - 1)
        logits = out_p.tile([1, V], F32, name=f"lg{b}", tag=f"lg{b}")
        nc.vector.tensor_add(out=logits[:], in0=pl[:], in1=bout_t[:])
        nmax = out_p.tile([1, 1], F32, name=f"nm{b}", tag=f"nm{b}")
        nc.vector.reduce_max(out=nmax[:], in_=logits[:], axis=AX.X, negate=True)
        den = out_p.tile([1, 1], F32, name=f"dn{b}", tag=f"dn{b}")
        nc.scalar.activation(
            out=logits[:], in_=logits[:], func=ACTF.Exp,
            bias=nmax[:, 0:1], accum_out=den[:],
        )
        rec = out_p.tile([1, 1], F32, name=f"rc{b}", tag=f"rc{b}")
        nc.vector.reciprocal(out=rec[:], in_=den[:])
        nc.vector.tensor_scalar_mul(out=logits[:], in0=logits[:], scalar1=rec[:])
        nc.sync.dma_start(out=probs[b : b + 1, :], in_=logits[:])


def _pe_table():
    i = np.arange(E, dtype=np.float32)
    rates = (1.0 / np.power(np.float32(10000.0), 2.0 * np.floor(i / 2.0) / E)).astype(
        np.float32
    )
    ang = np.arange(T, dtype=np.float32)[:, None] * rates[None, :]
    pe = np.concatenate([np.sin(ang[:, 0::2]), np.cos(ang[:, 1::2])], axis=-1)
    return np.tile(pe.astype(np.float32), (BPC, 1))  # [TOK, E]


def _maskd():
    j = np.arange(P)
    return np.where(j[None, :] < j[:, None], np.float32(NEG), np.float32(0.0))


def _prep_in_maps(
    input_tokens, emb, wq, wk, wv, wo, bo, ln1_w, ln1_b, ln2_w, ln2_b,
    ff1_w, ff1_b, ff2_w, ff2_b, wout, bout, fold_ln2=True,
):
    f = lambda x: np.ascontiguousarray(np.asarray(x, dtype=np.float32))
    w = lambda x: np.ascontiguousarray(np.asarray(x, dtype=np.float32).astype(NP_WDT))
    wq_ = np.asarray(wq, dtype=np.float32)
    wk_ = np.asarray(wk, dtype=np.float32)
    wv_ = np.asarray(wv, dtype=np.float32)
    wo_ = np.asarray(wo, dtype=np.float32).reshape(L, H, E, E)
    wqk = np.matmul(wq_, np.swapaxes(wk_, -1, -2))
    wvo = np.matmul(wv_, wo_)
    wout_ = np.asarray(wout, dtype=np.float32)
    ln2_w_ = np.asarray(ln2_w, dtype=np.float32)
    if fold_ln2:
        # x_{l+1} = LN2core(s2) * ln2_w (ln2_b == 0), so fold ln2_w[l] into
        # the next layer's Wqk (both sides) / Wvo (left side) and into wout.
        wqk = wqk.copy()
        wvo = wvo.copy()
        for l in range(1, L):
            s = ln2_w_[l - 1]
            wqk[l] = s[None, :, None] * wqk[l] * s[None, None, :]
            wvo[l] = s[None, :, None] * wvo[l]
        wout_ = ln2_w_[L - 1][:, None] * wout_
    # fold LN1's affine params into ff1 (exact):
    # relu((x*w1+b1) @ W + b) == relu(x @ (w1[:,None]*W) + (b1 @ W + b))
    ln1_w_ = np.asarray(ln1_w, dtype=np.float32)
    ln1_b_ = np.asarray(ln1_b, dtype=np.float32)
    ff1w_ = np.asarray(ff1_w, dtype=np.float32) * ln1_w_[:, :, None]
    ff1b_ = np.asarray(ff1_b, dtype=np.float32) + np.einsum(
        "le,leo->lo", ln1_b_, np.asarray(ff1_w, dtype=np.float32)
    )
    toks = np.asarray(input_tokens).astype(np.int64)
    shared = {
        "emb": f(emb), "wqk": w(wqk), "wvo": w(wvo),
        "bo": f(bo), "ln2w": f(ln2_w), "ln2b": f(ln2_b),
        "ff1w": w(ff1w_), "ff1b": f(ff1b_), "ff2w": w(ff2_w),
        "ff2b": f(ff2_b), "wout": w(wout_), "bout": f(bout),
        "pe2": w(_pe_table()), "maskd": _maskd(),
        "ident": np.eye(P, dtype=np.float32),
    }
    in_maps = []
    for c in range(NCORES):
        t = toks[c * BPC : (c + 1) * BPC].reshape(TOK)  # [512] flat tokens
        tokarr = np.ascontiguousarray(t.reshape(TC, P).T.astype(np.int32))
        in_maps.append({**shared, "tok": tokarr})
    return in_maps


def kernel(**inputs):
    ln2_b = np.asarray(inputs["ln2_b"], dtype=np.float32)
    fold = not np.any(ln2_b != 0.0)
    key = "nc" if fold else "nc_affine"
    if key not in _CACHE:
        _CACHE[key] = _build(dev_affine=not fold)
    nc = _CACHE[key]
    in_maps = _prep_in_maps(**inputs, fold_ln2=fold)
    res = run_bass_kernel_spmd(nc, in_maps, core_ids=list(range(NCORES)))
    _CACHE["last_results"] = res
    out = np.concatenate([res.results[c]["probs"] for c in range(NCORES)], axis=0)
    return out.astype(np.float32)
